# revision 1
# baseline (speedup 1.0000x reference)
"""DriftAwareGraphEncoder on 8 Trainium2 NeuronCores (Bass/Tile).

Strategy (matches the METIS-style sharding hint, specialized for a random
graph where the halo is ~all nodes):
  - Nodes partitioned contiguously across 8 cores (6250/core, padded to
    6272 = 49 windows x 128). Params replicated.
  - Per layer, each core computes a compact per-node table
    [hh (4x64) | gcn feat (64) | s_src (4)] for its shard, then an
    AllGather replicates the table (the halo exchange).
  - Edge work: per core, edges grouped by 128-dst windows; per-edge rows
    fetched with dma_gather (int16 indices -> two streams split at table
    row 32768); scatter-sum done on the tensor engine via 0/1 eq-matrices
    accumulating in PSUM (no scatter DMA). Softmax without max-subtraction
    (logits are O(1) here; alpha is mathematically identical).
  - BatchNorm in transposed form (features on partitions): stats via ACT
    accum + AllReduce; b1 dropped (BN is shift-invariant per feature).
"""

import os

import numpy as np

import concourse.bass as bass
import concourse.bacc as bacc
import concourse.mybir as mybir
import concourse.tile as tile
from concourse.bass_utils import run_bass_kernel_spmd
from concourse.masks import make_identity

# ---- problem constants (hardcoded per contract) ----
N = 50000
E0 = 600000
IN_DIM = 128
HID = 64
HEADS = 4
OUT_DIM = 64
EPS = 1e-5

NCORES = 8
NC_NODES = 6250          # real nodes per core
NPAD = 6272              # padded nodes per core (49 * 128)
NWIN = 49
TBL = 384                # table row width (cols; 324 used, 1536B fp32)
HALF = 32768             # int16 gather row split
BLK = 128                # edges per block
GCH = {0: 8, 1: 8}       # gather chunk size in blocks per stream (A, B)
PAD_DREL = 200.0         # dst_rel for padded edges (never matches 0..127)

F32 = mybir.dt.float32
BF16 = mybir.dt.bfloat16
I32 = mybir.dt.int32
I16 = mybir.dt.int16

_cache = {}


# ======================= host preprocessing =======================

def _build_plan_and_data(x, edge_index, W1, a_src1, a_dst1, gamma, beta,
                         W2, a_src2, a_dst2, b2, Wg1, bg1, Wg2, bg2,
                         drift_weight):
    src = np.concatenate([edge_index[0].astype(np.int64),
                          np.arange(N, dtype=np.int64)])
    dst = np.concatenate([edge_index[1].astype(np.int64),
                          np.arange(N, dtype=np.int64)])
    deg = np.bincount(dst, minlength=N).astype(np.float32)
    dinv = (1.0 / np.sqrt(np.maximum(deg, 1.0))).astype(np.float32)
    gw = dinv[src] * dinv[dst]

    row = (src // NC_NODES) * NPAD + (src % NC_NODES)   # table row id
    core = dst // NC_NODES
    ldst = dst - core * NC_NODES
    win = ldst // BLK
    drel = (ldst % BLK).astype(np.float32)
    stream = (row >= HALF).astype(np.int64)             # 0 = A, 1 = B

    # counts[c, s, w]
    counts = np.zeros((NCORES, 2, NWIN), np.int64)
    flat = (core * 2 + stream) * NWIN + win
    cb = np.bincount(flat, minlength=NCORES * 2 * NWIN)
    counts[:] = cb.reshape(NCORES, 2, NWIN)

    nblk = np.zeros((2, NWIN), np.int64)
    for s in range(2):
        nblk[s] = (counts[:, s, :].max(axis=0) + BLK - 1) // BLK
        nblk[s] = np.maximum(nblk[s], 1)

    tot_blk = [int(nblk[s].sum()) for s in range(2)]
    ncall = [ (tot_blk[s] * BLK + GCH[s] * BLK - 1) // (GCH[s] * BLK)
              for s in range(2) ]
    pad_blk = [ncall[s] * GCH[s] for s in range(2)]     # blocks incl junk tail
    nch128 = [(tot_blk[s] + 127) // 128 for s in range(2)]

    # windows -> runs of (stream, gb0, len); runs don't cross gather chunks
    blk0 = [np.concatenate([[0], np.cumsum(nblk[s])]).astype(np.int64)
            for s in range(2)]
    window_runs = []
    for w in range(NWIN):
        runs = []
        for s in range(2):
            g0, g1 = int(blk0[s][w]), int(blk0[s][w + 1])
            gb = g0
            while gb < g1:
                end = min(g1, ((gb // GCH[s]) + 1) * GCH[s])
                runs.append((s, gb, end - gb))
                gb = end
        window_runs.append(runs)

    plan = {
        "nblk": nblk, "tot_blk": tot_blk, "ncall": ncall,
        "pad_blk": pad_blk, "nch128": nch128, "window_runs": window_runs,
        "blk0": blk0,
    }

    # ---- per-core edge arrays ----
    per_core = []
    for c in range(NCORES):
        d = {}
        for s in range(2):
            tb = tot_blk[s]
            # pad rows must stay in this stream's index range after the
            # -HALF shift (stream B pad row 0 would go to int16 -32768!)
            rows_l = np.full(tb * BLK, HALF if s == 1 else 0, np.int64)
            drel_l = np.full(tb * BLK, PAD_DREL, np.float32)
            gw_l = np.zeros(tb * BLK, np.float32)
            sel_all = np.flatnonzero((core == c) & (stream == s))
            w_of = win[sel_all]
            order = np.lexsort((row[sel_all], w_of))
            sel_all = sel_all[order]
            w_sorted = w_of[order]
            starts = np.searchsorted(w_sorted, np.arange(NWIN))
            ends = np.searchsorted(w_sorted, np.arange(NWIN), side="right")
            for w in range(NWIN):
                seg = sel_all[starts[w]:ends[w]]
                o = int(blk0[s][w]) * BLK
                rows_l[o:o + len(seg)] = row[seg]
                drel_l[o:o + len(seg)] = drel[seg]
                gw_l[o:o + len(seg)] = gw[seg]
            # idx (int16), padded to gather calls, wrapped+replicated
            idx = rows_l - (HALF if s == 1 else 0)
            idx = np.concatenate([idx, np.zeros(pad_blk[s] * BLK - tb * BLK,
                                                np.int64)])
            a16 = idx.astype(np.int16).reshape(-1, 16).T    # [16, L/16]
            d[f"ix{s}"] = np.ascontiguousarray(np.tile(a16, (8, 1)))
            # dst_rel / gcn weight column layouts [128, tb]
            d[f"dr{s}"] = np.ascontiguousarray(
                drel_l.reshape(tb, BLK).T)
            d[f"gw{s}"] = np.ascontiguousarray(
                gw_l.reshape(tb, BLK).T)
        # x shard transposed + padded
        xs = x[c * NC_NODES:(c + 1) * NC_NODES]
        xT = np.zeros((IN_DIM, NPAD), np.float32)
        xT[:, :NC_NODES] = xs.T
        d["xT"] = xT
        per_core.append(d)

    # ---- shared (replicated) tensors ----
    def fold(W, a_s, a_d):
        WS = np.stack([W[:, h * HID:(h + 1) * HID] @ a_s[h]
                       for h in range(HEADS)], axis=1)
        WD = np.stack([W[:, h * HID:(h + 1) * HID] @ a_d[h]
                       for h in range(HEADS)], axis=1)
        return WS.astype(np.float32), WD.astype(np.float32)

    W1S, W1D = fold(W1, a_src1, a_dst1)
    W2S, W2D = fold(W2, a_src2, a_dst2)
    shared = {
        "Wcat1": np.concatenate([W1, Wg1, W1S, W1D], axis=1).astype(np.float32),
        "Wcat2": np.concatenate([W2, W2S, W2D], axis=1).astype(np.float32),
        "Wg2": Wg2.astype(np.float32),
        "gb4": np.stack([gamma[:128], gamma[128:], beta[:128], beta[128:]],
                        axis=1).astype(np.float32),
        "bg1c": bg1.reshape(HID, 1).astype(np.float32),
        "b2r": np.tile(b2.reshape(1, OUT_DIM), (128, 1)).astype(np.float32),
        "bg2r": np.tile(bg2.reshape(1, OUT_DIM), (128, 1)).astype(np.float32),
        "dw": np.tile(drift_weight.reshape(1, 1), (128, 1)).astype(np.float32),
    }
    return plan, per_core, shared


# ======================= device kernel =======================

def build_kernel(plan, debug=False, upto=4):
    EW = int(os.environ.get("KERNEL_EW", str(NWIN)))
    NO_SDE = os.environ.get("KERNEL_NO_SDE", "0") == "1"
    NO_GATHER = os.environ.get("KERNEL_NO_GATHER", "0") == "1"
    NO_CC = os.environ.get("KERNEL_NO_CC", "0") == "1"
    # upto: 1 = phase A + AG1, 2 = + edge pass 1, 3 = + BN/phase B + AG2,
    #       4 = full
    nc = bacc.Bacc("TRN2", target_bir_lowering=False, debug=False,
                   num_devices=NCORES, dynamic_dma_scratch_size=32768,
                   num_swdge_queues=4)
    AL = mybir.AluOpType
    AF = mybir.ActivationFunctionType
    nblk, tot_blk, ncall = plan["nblk"], plan["tot_blk"], plan["ncall"]
    nch128, window_runs = plan["nch128"], plan["window_runs"]

    # ---- I/O ----
    ins = {}
    ins["xT"] = nc.dram_tensor("xT", [IN_DIM, NPAD], F32, kind="ExternalInput")
    for s in range(2):
        L = plan["pad_blk"][s] * BLK
        ins[f"ix{s}"] = nc.dram_tensor(f"ix{s}", [128, L // 16], I16,
                                       kind="ExternalInput")
        ins[f"dr{s}"] = nc.dram_tensor(f"dr{s}", [128, tot_blk[s]], F32,
                                       kind="ExternalInput")
        ins[f"gw{s}"] = nc.dram_tensor(f"gw{s}", [128, tot_blk[s]], F32,
                                       kind="ExternalInput")
    ins["Wcat1"] = nc.dram_tensor("Wcat1", [IN_DIM, 328], F32, kind="ExternalInput")
    ins["Wcat2"] = nc.dram_tensor("Wcat2", [256, 264], F32, kind="ExternalInput")
    ins["Wg2"] = nc.dram_tensor("Wg2", [HID, HID], F32, kind="ExternalInput")
    ins["gb4"] = nc.dram_tensor("gb4", [128, 4], F32, kind="ExternalInput")
    ins["bg1c"] = nc.dram_tensor("bg1c", [HID, 1], F32, kind="ExternalInput")
    ins["b2r"] = nc.dram_tensor("b2r", [128, OUT_DIM], F32, kind="ExternalInput")
    ins["bg2r"] = nc.dram_tensor("bg2r", [128, OUT_DIM], F32, kind="ExternalInput")
    ins["dw"] = nc.dram_tensor("dw", [128, 1], F32, kind="ExternalInput")

    out_t = nc.dram_tensor("out", [NPAD, OUT_DIM], F32, kind="ExternalOutput")
    dbg = {}
    if debug:
        for nm, shp in [("dbg_t1", [NPAD, TBL]), ("dbg_t2", [NPAD, TBL]),
                        ("dbg_stats", [128, 4]), ("dbg_h1T", [256, NPAD]),
                        ("dbg_act1T", [256, NPAD]), ("dbg_g1T", [HID, NPAD])]:
            dbg[nm] = nc.dram_tensor(nm, shp, F32, kind="ExternalOutput")

    with tile.TileContext(nc) as tc:
        with (
            tc.tile_pool(name="cst", bufs=1) as cst,
            tc.tile_pool(name="meta", bufs=1) as meta,
            tc.tile_pool(name="resid", bufs=1) as resid,
            tc.tile_pool(name="dram", bufs=1, space="DRAM") as dram,
        ):
            # ---- constants ----
            ident = cst.tile([128, 128], F32)
            make_identity(nc, ident[:])
            io_fi = cst.tile([128, 8 * 128], I32)
            nc.gpsimd.iota(io_fi[:], pattern=[[0, 8], [1, 128]], base=0,
                           channel_multiplier=0)
            iota_free = cst.tile([128, 8 * 128], F32)
            nc.vector.tensor_copy(out=iota_free[:], in_=io_fi[:])
            io_pi = cst.tile([128, 128], I32)
            nc.gpsimd.iota(io_pi[:], pattern=[[0, 128]], base=0,
                           channel_multiplier=1)
            iota_part = cst.tile([128, 128], F32)
            nc.vector.tensor_copy(out=iota_part[:], in_=io_pi[:])

            # ---- metadata loads ----
            ix_sb, dr_sb, gw_sb = {}, {}, {}
            for s in range(2):
                L = plan["pad_blk"][s] * BLK
                ix_sb[s] = meta.tile([128, L // 16], I16, name=f"ixt{s}")
                nc.sync.dma_start(out=ix_sb[s][:], in_=ins[f"ix{s}"][:])
                dr_sb[s] = meta.tile([128, tot_blk[s]], F32, name=f"drt{s}")
                nc.sync.dma_start(out=dr_sb[s][:], in_=ins[f"dr{s}"][:])
                gw_sb[s] = meta.tile([128, tot_blk[s]], F32, name=f"gwt{s}")
                nc.sync.dma_start(out=gw_sb[s][:], in_=ins[f"gw{s}"][:])

            wc2 = [meta.tile([128, 264], F32, name=f"wc2_{k}") for k in range(2)]
            for k in range(2):
                nc.sync.dma_start(out=wc2[k][:], in_=ins["Wcat2"][k * 128:(k + 1) * 128, :])
            wg2t = meta.tile([HID, HID], F32)
            nc.sync.dma_start(out=wg2t[:], in_=ins["Wg2"][:])
            gb4t = meta.tile([128, 4], F32)
            nc.sync.dma_start(out=gb4t[:], in_=ins["gb4"][:])
            bg1t = meta.tile([HID, 1], F32)
            nc.sync.dma_start(out=bg1t[:], in_=ins["bg1c"][:])
            b2t = meta.tile([128, OUT_DIM], F32)
            nc.sync.dma_start(out=b2t[:], in_=ins["b2r"][:])
            bg2t = meta.tile([128, OUT_DIM], F32)
            nc.sync.dma_start(out=bg2t[:], in_=ins["bg2r"][:])
            wb = meta.tile([128, 1], F32)
            nc.sync.dma_start(out=wb[:], in_=ins["dw"][:])

            # ---- residents ----
            h1T = ([resid.tile([128, NPAD], F32, name=f"h1T{k}")
                    for k in range(2)] if upto >= 2 else None)
            sdst = [resid.tile([128, NWIN * 4], F32, name="sdst0")]
            if upto >= 3:
                sdst.append(resid.tile([128, NWIN * 4], F32, name="sdst1"))

            # ---- DRAM internals ----
            Tsh = [dram.tile([NPAD, TBL], BF16, name=f"Tsh{k}") for k in range(2)]
            Tfull = [dram.tile([NCORES * NPAD, TBL], BF16, addr_space="Shared",
                               name=f"Tfull{k}") for k in range(2)]
            g1T_d = dram.tile([HID, NPAD], F32)
            st_in = dram.tile([128, 4], F32)
            st_out = dram.tile([128, 4], F32, addr_space="Shared")

            rg = [list(range(NCORES))]

            REPS = int(os.environ.get("KERNEL_REPS", "1"))
            for _rep in range(REPS):
                # ================= phase A =================
                with (
                    tc.tile_pool(name="pa", bufs=1) as pa,
                    tc.tile_pool(name="psA", bufs=2, space="PSUM") as psA,
                    tc.tile_pool(name="wa", bufs=3) as wa,
                ):
                    xTt = pa.tile([IN_DIM, NPAD], F32)
                    nc.sync.dma_start(out=xTt[:], in_=ins["xT"][:])
                    wc1 = pa.tile([IN_DIM, 328], F32)
                    nc.sync.dma_start(out=wc1[:], in_=ins["Wcat1"][:])
                    for i in range(NWIN):
                        ps = psA.tile([128, 328], F32, tag="psA")
                        nc.tensor.matmul(out=ps[:], lhsT=xTt[:, i * 128:(i + 1) * 128],
                                         rhs=wc1[:], start=True, stop=True)
                        t1 = wa.tile([128, TBL], BF16, tag="t1")
                        nc.vector.tensor_copy(out=t1[:, 0:324], in_=ps[:, 0:324])
                        nc.vector.memset(t1[:, 324:TBL], 0.0)
                        nc.vector.tensor_copy(out=sdst[0][:, i * 4:(i + 1) * 4],
                                              in_=ps[:, 324:328])
                        nc.sync.dma_start(out=Tsh[0][i * 128:(i + 1) * 128, :],
                                          in_=t1[:])

                if NO_CC:
                    nc.gpsimd.dma_start(out=Tfull[0][0:NPAD, :], in_=Tsh[0][:])
                else:
                    nc.gpsimd.collective_compute(
                        "AllGather", mybir.AluOpType.bypass, replica_groups=rg,
                        ins=[Tsh[0].opt()], outs=[Tfull[0].opt()])
                if debug:
                    _dma_dram(nc, dbg["dbg_t1"], Tsh[0])

                # ================= phase 1 (edge pass) =================
                def edge_pass(ph, Tf, sdst_sb, post_window):
                    with (
                        tc.tile_pool(name=f"mg{ph}", bufs=1) as mgp,
                        tc.tile_pool(name=f"wp{ph}", bufs=1) as wp,
                        tc.tile_pool(name=f"pse{ph}", bufs=2, space="PSUM") as pse,
                        tc.tile_pool(name=f"pst{ph}", bufs=2, space="PSUM") as pst,
                    ):
                        chunk_tiles = {}

                        def get_chunk(s, ci):
                            key = (s, ci)
                            if key not in chunk_tiles:
                                tf = mgp.tile([128, GCH[s] * TBL], F32,
                                              tag=f"mf{s}", bufs=2,
                                              name=f"mf{ph}_{s}_{ci}")
                                if NO_GATHER:
                                    nc.vector.memset(tf[:], 0.5)
                                    chunk_tiles[key] = tf
                                    return tf
                                t = mgp.tile([128, GCH[s] * TBL], BF16,
                                             tag=f"mg{s}", bufs=2 if s == 0 else 1,
                                             name=f"mg{ph}_{s}_{ci}")
                                nidx = GCH[s] * BLK
                                base = Tf[:] if s == 0 else Tf[HALF:, :]
                                nc.gpsimd.dma_gather(
                                    out_ap=t[:].rearrange("p (n d) -> p n d", d=TBL),
                                    in_ap=base,
                                    idxs_ap=ix_sb[s][:, ci * (nidx // 16):(ci + 1) * (nidx // 16)],
                                    num_idxs=nidx, num_idxs_reg=nidx,
                                    elem_size=TBL, queue_num=(2 * ci + s) % 4)
                                nc.vector.tensor_copy(out=tf[:], in_=t[:])
                                chunk_tiles[key] = tf
                            return chunk_tiles[key]

                        for w in range(EW):
                            runs = window_runs[w]
                            nb_w = sum(r[2] for r in runs)
                            out_ps = pse.tile([128, 324], F32, tag="outw", bufs=2)
                            sde_ps = pse.tile([128, 4 * nb_w], F32, tag="sde", bufs=2)
                            sw = sdst_sb[:, w * 4:(w + 1) * 4]
                            j = 0
                            last_j = nb_w - 1
                            for (s, gb0, ln) in runs:
                                ci = gb0 // GCH[s]
                                sl0 = gb0 % GCH[s]
                                mg = get_chunk(s, ci)
                                M3 = mg[:].rearrange("p (l c) -> p l c", c=TBL)
                                # batched eqT: eqT[e, l, i] = (drel[e, l] == i)
                                eqT = wp.tile([128, ln * 128], F32, tag="eqT", bufs=2)
                                nc.vector.tensor_tensor(
                                    out=eqT[:].rearrange("p (l i) -> p l i", i=128),
                                    in0=iota_free[:, 0:ln * 128]
                                        .rearrange("p (l i) -> p l i", i=128),
                                    in1=dr_sb[s][:, gb0:gb0 + ln].unsqueeze(2)
                                        .to_broadcast([128, ln, 128]),
                                    op=AL.is_equal)
                                # p chain (batched over run)
                                v = wp.tile([128, ln * 4], F32, tag="v", bufs=3)
                                if NO_SDE:
                                    nc.vector.tensor_copy(
                                        out=v[:].rearrange("p (l c) -> p l c", c=4),
                                        in_=M3[:, sl0:sl0 + ln, 320:324])
                                else:
                                    # eq = eqT^T per block (PE transpose), then sde
                                    eq_sb = wp.tile([128, ln * 128], F32,
                                                    tag="eqsb", bufs=2)
                                    for k0 in range(0, ln, 4):
                                        kn = min(4, ln - k0)
                                        etp = pst.tile([128, kn * 128], F32,
                                                       tag="eqtr", bufs=2)
                                        for k in range(kn):
                                            nc.tensor.transpose(
                                                out=etp[:, k * 128:(k + 1) * 128],
                                                in_=eqT[:, (k0 + k) * 128:(k0 + k + 1) * 128],
                                                identity=ident[:])
                                        nc.vector.tensor_copy(
                                            out=eq_sb[:, k0 * 128:(k0 + kn) * 128],
                                            in_=etp[:])
                                    for k in range(ln):
                                        nc.tensor.matmul(
                                            out=sde_ps[:, (j + k) * 4:(j + k + 1) * 4],
                                            lhsT=eq_sb[:, k * 128:(k + 1) * 128],
                                            rhs=sw, start=True, stop=True)
                                    nc.vector.tensor_tensor(
                                        out=v[:].rearrange("p (l c) -> p l c", c=4),
                                        in0=M3[:, sl0:sl0 + ln, 320:324],
                                        in1=sde_ps[:, j * 4:(j + ln) * 4]
                                            .rearrange("p (l c) -> p l c", c=4),
                                        op=AL.add)
                                lr = wp.tile([128, ln * 4], F32, tag="lr", bufs=3)
                                nc.vector.tensor_scalar_mul(out=lr[:], in0=v[:],
                                                            scalar1=0.2)
                                nc.vector.tensor_tensor(out=lr[:], in0=v[:],
                                                        in1=lr[:], op=AL.max)
                                p = wp.tile([128, ln * 4], F32, tag="p", bufs=3)
                                nc.scalar.activation(out=p[:], in_=lr[:], func=AF.Exp)
                                # folds -> rhs
                                rhs = wp.tile([128, ln * 324], F32, tag="rhs", bufs=2)
                                R3 = rhs[:].rearrange("p (l c) -> p l c", c=324)
                                p3 = p[:].rearrange("p (l c) -> p l c", c=4)
                                nc.vector.tensor_tensor(
                                    out=R3[:, :, 0:256]
                                        .rearrange("p l (h c) -> p l h c", c=HID),
                                    in0=M3[:, sl0:sl0 + ln, 0:256]
                                        .rearrange("p l (h c) -> p l h c", c=HID),
                                    in1=p3.unsqueeze(3).to_broadcast([128, ln, 4, HID]),
                                    op=AL.mult)
                                nc.vector.tensor_copy(out=R3[:, :, 256:260], in_=p3)
                                nc.vector.tensor_tensor(
                                    out=R3[:, :, 260:324],
                                    in0=M3[:, sl0:sl0 + ln, 256:320],
                                    in1=gw_sb[s][:, gb0:gb0 + ln].unsqueeze(2)
                                        .to_broadcast([128, ln, HID]),
                                    op=AL.mult)
                                # main matmuls
                                for k in range(ln):
                                    nc.tensor.matmul(
                                        out=out_ps[:],
                                        lhsT=eqT[:, k * 128:(k + 1) * 128],
                                        rhs=rhs[:, k * 324:(k + 1) * 324],
                                        start=(j + k == 0), stop=(j + k == last_j))
                                j += ln
                            post_window(w, out_ps, wp, pst)

                # ---- phase-1 per-window epilogue ----
                def post1(w, out_ps, wp, pst):
                    zc = wp.tile([128, 4], F32, tag="zc", bufs=2)
                    nc.vector.tensor_scalar_max(out=zc[:], in0=out_ps[:, 256:260],
                                                scalar1=1e-30)
                    zr = wp.tile([128, 4], F32, tag="zr", bufs=2)
                    nc.vector.reciprocal(out=zr[:], in_=zc[:])
                    h1n = wp.tile([128, 256], F32, tag="h1n", bufs=2)
                    for h in range(HEADS):
                        nc.vector.tensor_scalar_mul(
                            out=h1n[:, h * HID:(h + 1) * HID],
                            in0=out_ps[:, h * HID:(h + 1) * HID],
                            scalar1=zr[:, h:h + 1])
                    # transpose h1n -> h1T halves
                    for k in range(2):
                        tp = pst.tile([128, 128], F32, tag="tp", bufs=2)
                        nc.tensor.transpose(out=tp[:], in_=h1n[:, k * 128:(k + 1) * 128],
                                            identity=ident[:])
                        nc.vector.tensor_copy(out=h1T[k][:, w * 128:(w + 1) * 128],
                                              in_=tp[:])
                    # gcn1: copy to SBUF, transpose, then bias+relu on ACT
                    g1s = wp.tile([128, HID], F32, tag="g1s", bufs=2)
                    nc.vector.tensor_copy(out=g1s[:], in_=out_ps[:, 260:324])
                    tp2 = pst.tile([128, 128], F32, tag="tp", bufs=2)
                    nc.tensor.transpose(out=tp2[0:HID, :], in_=g1s[:],
                                        identity=ident[:])
                    g1c = wp.tile([HID, 128], F32, tag="g1c", bufs=2)
                    nc.scalar.activation(out=g1c[:], in_=tp2[0:HID, :],
                                         func=mybir.ActivationFunctionType.Relu,
                                         bias=bg1t[:, 0:1], scale=1.0)
                    nc.sync.dma_start(out=g1T_d[:, w * 128:(w + 1) * 128], in_=g1c[:])

                if upto >= 2:
                    if EW < NWIN:
                        for k in range(2):
                            nc.vector.memset(h1T[k][:], 0.0)
                    edge_pass(1, Tfull[0], sdst[0], post1)
                if upto == 2 and debug:
                    for k in range(2):
                        nc.sync.dma_start(
                            out=dbg["dbg_h1T"][k * 128:(k + 1) * 128, :],
                            in_=h1T[k][:])

                # ================= BN + ELU + phase B =================
                AF = mybir.ActivationFunctionType
                if upto >= 3:
                    with (
                        tc.tile_pool(name="bn", bufs=1) as bn,
                        tc.tile_pool(name="psB", bufs=2, space="PSUM") as psB,
                        tc.tile_pool(name="wb2", bufs=3) as wb2,
                    ):
                        # stats
                        st = bn.tile([128, 4], F32)
                        scratch = bn.tile([128, NPAD], F32)
                        for k in range(2):
                            nc.scalar.activation(out=scratch[:], in_=h1T[k][:],
                                                 func=AF.Identity,
                                                 accum_out=st[:, k:k + 1])
                            nc.scalar.activation(out=scratch[:], in_=h1T[k][:],
                                                 func=AF.Square,
                                                 accum_out=st[:, 2 + k:3 + k])
                        nc.sync.dma_start(out=st_in[:], in_=st[:])
                        if NO_CC:
                            nc.gpsimd.dma_start(out=st_out[:], in_=st_in[:])
                        else:
                            nc.gpsimd.collective_compute(
                                "AllReduce", mybir.AluOpType.add, replica_groups=rg,
                                ins=[st_in.opt()], outs=[st_out.opt()])
                        stg = bn.tile([128, 4], F32)
                        nc.sync.dma_start(out=stg[:], in_=st_out[:])
                        if debug:
                            nc.sync.dma_start(out=dbg["dbg_stats"][:], in_=stg[:])
                            for k in range(2):
                                nc.sync.dma_start(
                                    out=dbg["dbg_h1T"][k * 128:(k + 1) * 128, :],
                                    in_=h1T[k][:])

                        inv_n = 1.0 / float(N)
                        mean = bn.tile([128, 2], F32)
                        nc.vector.tensor_scalar_mul(out=mean[:], in0=stg[:, 0:2],
                                                    scalar1=inv_n)
                        var = bn.tile([128, 2], F32)
                        nc.vector.tensor_scalar_mul(out=var[:], in0=stg[:, 2:4],
                                                    scalar1=inv_n)
                        musq = bn.tile([128, 2], F32)
                        nc.vector.tensor_tensor(out=musq[:], in0=mean[:], in1=mean[:],
                                                op=mybir.AluOpType.mult)
                        nc.vector.tensor_tensor(out=var[:], in0=var[:], in1=musq[:],
                                                op=mybir.AluOpType.subtract)
                        epst = bn.tile([128, 1], F32)
                        nc.vector.memset(epst[:], float(EPS))
                        sd = bn.tile([128, 2], F32)
                        nc.scalar.activation(out=sd[:], in_=var[:], func=AF.Sqrt,
                                             bias=epst[:, 0:1], scale=1.0)
                        rstd = bn.tile([128, 2], F32)
                        nc.vector.reciprocal(out=rstd[:], in_=sd[:])
                        scl = bn.tile([128, 2], F32)
                        nc.vector.tensor_tensor(out=scl[:], in0=rstd[:], in1=gb4t[:, 0:2],
                                                op=mybir.AluOpType.mult)
                        shf = bn.tile([128, 2], F32)
                        nc.vector.tensor_tensor(out=shf[:], in0=mean[:], in1=scl[:],
                                                op=mybir.AluOpType.mult)
                        nc.vector.tensor_tensor(out=shf[:], in0=gb4t[:, 2:4], in1=shf[:],
                                                op=mybir.AluOpType.subtract)

                        # BN + ELU in transposed form, in place on h1T
                        sc2 = bn.tile([128, NPAD], F32)
                        for k in range(2):
                            nc.scalar.activation(out=scratch[:], in_=h1T[k][:],
                                                 func=AF.Identity,
                                                 bias=shf[:, k:k + 1],
                                                 scale=scl[:, k:k + 1])
                            nc.vector.tensor_scalar_min(out=sc2[:], in0=scratch[:],
                                                        scalar1=0.0)
                            nc.scalar.activation(out=sc2[:], in_=sc2[:], func=AF.Exp)
                            nc.vector.tensor_scalar_max(out=scratch[:], in0=scratch[:],
                                                        scalar1=0.0)
                            nc.vector.tensor_tensor(out=scratch[:], in0=scratch[:],
                                                    in1=sc2[:], op=mybir.AluOpType.add)
                            nc.vector.tensor_scalar_sub(out=h1T[k][:], in0=scratch[:],
                                                        scalar1=1.0)
                        if debug:
                            for k in range(2):
                                nc.sync.dma_start(
                                    out=dbg["dbg_act1T"][k * 128:(k + 1) * 128, :],
                                    in_=h1T[k][:])

                        # phase B node matmuls
                        for i in range(NWIN):
                            ps = psB.tile([128, 328], F32, tag="psB", bufs=2)
                            nc.tensor.matmul(out=ps[:, 0:264],
                                             lhsT=h1T[0][:, i * 128:(i + 1) * 128],
                                             rhs=wc2[0][:], start=True, stop=False)
                            nc.tensor.matmul(out=ps[:, 0:264],
                                             lhsT=h1T[1][:, i * 128:(i + 1) * 128],
                                             rhs=wc2[1][:], start=False, stop=True)
                            g1c = wb2.tile([HID, 128], F32, tag="g1l", bufs=2)
                            nc.sync.dma_start(out=g1c[:], in_=g1T_d[:, i * 128:(i + 1) * 128])
                            nc.tensor.matmul(out=ps[:, 264:328], lhsT=g1c[:],
                                             rhs=wg2t[:], start=True, stop=True)
                            t2 = wb2.tile([128, TBL], BF16, tag="t2", bufs=3)
                            nc.vector.tensor_copy(out=t2[:, 0:256], in_=ps[:, 0:256])
                            nc.vector.tensor_copy(out=t2[:, 256:320], in_=ps[:, 264:328])
                            nc.vector.tensor_copy(out=t2[:, 320:324], in_=ps[:, 256:260])
                            nc.vector.memset(t2[:, 324:TBL], 0.0)
                            nc.vector.tensor_copy(out=sdst[1][:, i * 4:(i + 1) * 4],
                                                  in_=ps[:, 260:264])
                            nc.sync.dma_start(out=Tsh[1][i * 128:(i + 1) * 128, :],
                                              in_=t2[:])
                        if debug:
                            nc.gpsimd.dma_start(out=dbg["dbg_g1T"][:], in_=g1T_d[:])

                if upto >= 3:
                    if NO_CC:
                        nc.gpsimd.dma_start(out=Tfull[1][0:NPAD, :], in_=Tsh[1][:])
                    else:
                        nc.gpsimd.collective_compute(
                            "AllGather", mybir.AluOpType.bypass, replica_groups=rg,
                            ins=[Tsh[1].opt()], outs=[Tfull[1].opt()])
                    if debug:
                        _dma_dram(nc, dbg["dbg_t2"], Tsh[1])

                # ================= phase 2 (edge pass) =================
                def post2(w, out_ps, wp, pst):
                    zc = wp.tile([128, 4], F32, tag="zc", bufs=2)
                    nc.vector.tensor_scalar_max(out=zc[:], in0=out_ps[:, 256:260],
                                                scalar1=1e-30)
                    zr = wp.tile([128, 4], F32, tag="zr", bufs=2)
                    nc.vector.reciprocal(out=zr[:], in_=zc[:])
                    zr4 = wp.tile([128, 4], F32, tag="zr4", bufs=2)
                    nc.vector.tensor_scalar_mul(out=zr4[:], in0=zr[:], scalar1=0.25)
                    acc = wp.tile([128, OUT_DIM], F32, tag="acc", bufs=2)
                    tmp = wp.tile([128, OUT_DIM], F32, tag="tmp", bufs=2)
                    nc.vector.tensor_scalar_mul(out=acc[:], in0=out_ps[:, 0:HID],
                                                scalar1=zr4[:, 0:1])
                    for h in range(1, HEADS):
                        nc.vector.tensor_scalar_mul(
                            out=tmp[:], in0=out_ps[:, h * HID:(h + 1) * HID],
                            scalar1=zr4[:, h:h + 1])
                        nc.vector.tensor_tensor(out=acc[:], in0=acc[:], in1=tmp[:],
                                                op=mybir.AluOpType.add)
                    nc.vector.tensor_tensor(out=acc[:], in0=acc[:], in1=b2t[:],
                                            op=mybir.AluOpType.add)
                    gcn = wp.tile([128, OUT_DIM], F32, tag="gcn", bufs=2)
                    nc.vector.tensor_tensor(out=gcn[:], in0=out_ps[:, 260:324],
                                            in1=bg2t[:], op=mybir.AluOpType.add)
                    # out = gcn + w*(gat - gcn)
                    nc.vector.tensor_tensor(out=acc[:], in0=acc[:], in1=gcn[:],
                                            op=mybir.AluOpType.subtract)
                    nc.vector.tensor_scalar_mul(out=acc[:], in0=acc[:],
                                                scalar1=wb[:, 0:1])
                    nc.vector.tensor_tensor(out=acc[:], in0=acc[:], in1=gcn[:],
                                            op=mybir.AluOpType.add)
                    nc.sync.dma_start(out=out_t[w * 128:(w + 1) * 128, :], in_=acc[:])

                if upto >= 4:
                    edge_pass(2, Tfull[1], sdst[1], post2)

    nc.compile()
    return nc


def _dma_dram(nc, dst, src):
    # DRAM->DRAM copy (debug only); SWDGE handles DRAM->DRAM
    nc.gpsimd.dma_start(out=dst[:], in_=src[:])


# ======================= entry point =======================

def kernel(**inputs):
    x = np.asarray(inputs["x"], np.float32)
    edge_index = np.asarray(inputs["edge_index"])
    plan, per_core, shared = _build_plan_and_data(
        x, edge_index,
        np.asarray(inputs["W1"], np.float32), np.asarray(inputs["a_src1"], np.float32),
        np.asarray(inputs["a_dst1"], np.float32), np.asarray(inputs["gamma"], np.float32),
        np.asarray(inputs["beta"], np.float32), np.asarray(inputs["W2"], np.float32),
        np.asarray(inputs["a_src2"], np.float32), np.asarray(inputs["a_dst2"], np.float32),
        np.asarray(inputs["b2"], np.float32), np.asarray(inputs["Wg1"], np.float32),
        np.asarray(inputs["bg1"], np.float32), np.asarray(inputs["Wg2"], np.float32),
        np.asarray(inputs["bg2"], np.float32), np.asarray(inputs["drift_weight"], np.float32),
    )
    key = ("k", plan["tot_blk"][0], plan["tot_blk"][1],
           plan["pad_blk"][0], plan["pad_blk"][1])
    debug = bool(int(os.environ.get("KERNEL_DEBUG", "0")))
    upto = int(os.environ.get("KERNEL_UPTO", "4"))
    key = key + (debug, upto)
    if key not in _cache:
        _cache[key] = build_kernel(plan, debug=debug, upto=upto)
    nc = _cache[key]
    in_maps = [{**shared, **per_core[c]} for c in range(NCORES)]
    res = run_bass_kernel_spmd(nc, in_maps, list(range(NCORES)))
    out = np.concatenate([res.results[c]["out"][:NC_NODES]
                          for c in range(NCORES)], axis=0)
    if debug:
        kernel.last_results = res
    return out

